# revision 1
# baseline (speedup 1.0000x reference)
"""Circle Loss (PML-style) on 8 Trainium2 NeuronCores via Bass/Tile.

Full inputs -> full scalar output. Row-sharded: each core computes the
per-row masked logsumexps for a block of 1024 rows of the 8192x8192
cosine-similarity matrix; host does normalization, sharding, and the
final nonzero-mean reduction.

Math (gamma=256, m=0.25, OP=1.25, ON=-0.25, dP=0.75, dN=0.25):
  fp = -g*relu(OP-D)*(D-dP) = g*((D-1)^2 - 1/16)        (D<=1 => relu inactive)
  fn =  g*relu(D-ON)*(D-dN) = g*((relu(D+1/4))^2 - relu(D+1/4)/2)
  new = pos*fp + neg*fn
  lse_p = masked_logsumexp(new, pos); lse_n = masked_logsumexp(new, neg)
  loss_row = softplus(lse_p + lse_n); mean over rows with loss>0 (valid rows)

Device works in units of h = new/gamma, shifted by B so the
mask-multiplied tensor separates real entries (>= B-1/8 > 0) from
masked-out zeros; exp applies scale=256 with per-row bias = -256*max,
so B cancels exactly. fp16 intermediates keep DVE ops in 2x mode
(+-0.5 logit-unit rounding, which averages out over 8192 rows).
"""

import sys

sys.path.insert(0, "/opt/trn_rl_repo")

import numpy as np

TWO_N = 8192
D_EMB = 256
N_CORES = 8
ROWS_PER_CORE = TWO_N // N_CORES  # 1024
RT = ROWS_PER_CORE // 128  # 8 row tiles per core
CHUNK = 2048
NCH = TWO_N // CHUNK  # 4 column chunks
B_SHIFT = 0.75
# fraction of chunks whose neg-image goes through ACT (Square) instead of
# the DVE route; balances the two engines
ACT_ROUTE = (True, False, True, False)

_RUN_KWARGS: dict = {}
_NC_CACHE: dict = {}


def _split_waits(nc, maxw=1):
    """walrus in this container accepts at most ~2 sem-waits per
    instruction (1 for ACTIVATE); split extras onto preceding NoOps."""
    import concourse.mybir as mybir

    n_new = 0
    for bb in nc.main_func.blocks:
        insts = bb.instructions
        i = 0
        while i < len(insts):
            ins = insts[i]
            si = ins.sync_info
            if si is not None and si.on_wait and len(si.on_wait) > maxw:
                waits = list(si.on_wait)
                ins.sync_info = mybir.SyncInfo(
                    on_wait=waits[:maxw], on_update=si.on_update
                )
                rest = waits[maxw:]
                pos = i
                while rest:
                    chunk, rest = rest[:maxw], rest[maxw:]
                    nop = mybir.InstNoOp(name=f"I-waitfix-{n_new}")
                    n_new += 1
                    nop.engine = ins.engine
                    nop.sync_info = mybir.SyncInfo(on_wait=chunk, on_update=[])
                    insts.insert(pos, nop)
                    pos += 1
                    i += 1
            i += 1
    return n_new


def _build_nc(disjoint=False):
    import os
    import concourse.bass as bass
    import concourse.tile as tile
    from concourse import mybir

    no_inplace = os.environ.get("K_NOINPLACE", "0") == "1"
    no_gp = os.environ.get("K_NOGP", "0") == "1"
    no_exp = os.environ.get("K_NOEXP", "0") == "1"
    repeat = int(os.environ.get("K_REPEAT", "1"))

    f32 = mybir.dt.float32
    f16 = mybir.dt.float16
    AF = mybir.ActivationFunctionType
    ALU = mybir.AluOpType

    nc = bass.Bass("TRN2", target_bir_lowering=False)

    def reg_const(val, dtype=f32):
        t = nc.alloc_sbuf_tensor(f"const-{dtype.name}-{val}", [128, 1], dtype)
        nc.gpsimd.memset(t.ap(), val)
        nc.const_aps.aps[(dtype, val)] = t.ap()

    for vv in (-1.0, 0.25, -0.25):
        reg_const(vv)
    nc.all_engine_barrier()

    eT = nc.dram_tensor("eT", [D_EMB, TWO_N], f16, kind="ExternalInput")
    erT = nc.dram_tensor("erT", [D_EMB, ROWS_PER_CORE], f16, kind="ExternalInput")
    posm = nc.dram_tensor("posm", [ROWS_PER_CORE, TWO_N], f16, kind="ExternalInput")
    negm = nc.dram_tensor("negm", [ROWS_PER_CORE, TWO_N], f16, kind="ExternalInput")
    loss_out = nc.dram_tensor("loss", [128, RT], f32, kind="ExternalOutput")

    with tile.TileContext(nc) as tc:
        with tc.tile_pool(name="singles", bufs=1) as singles, \
             tc.tile_pool(name="chunks", bufs=3) as chunks, \
             tc.tile_pool(name="masks", bufs=2) as maskp, \
             tc.tile_pool(name="arow", bufs=6) as arowp, \
             tc.tile_pool(name="rmax", bufs=2) as rmaxp, \
             tc.tile_pool(name="small", bufs=4) as small, \
             tc.tile_pool(name="psum", bufs=2, space="PSUM") as psump:

            e_sb = []
            er_sb = []
            for k in range(2):
                t = singles.tile([128, TWO_N], f16, tag=f"e{k}")
                nc.sync.dma_start(out=t, in_=eT[k * 128:(k + 1) * 128, :])
                e_sb.append(t)
                tr = singles.tile([128, ROWS_PER_CORE], f16, tag=f"er{k}")
                nc.sync.dma_start(out=tr, in_=erT[k * 128:(k + 1) * 128, :])
                er_sb.append(tr)

            sp_all = singles.tile([128, RT], f32, tag="sp_all")
            sn_all = singles.tile([128, RT], f32, tag="sn_all")
            mp_all = singles.tile([128, RT], f32, tag="mp_all")
            mn_all = singles.tile([128, RT], f32, tag="mn_all")

            for rep in range(repeat):
              for rt in range(RT):
                r0 = rt * 128
                ap_cs = []
                an_cs = []
                rmp = rmaxp.tile([128, CHUNK], f16, tag="rmp")
                rmn = rmaxp.tile([128, CHUNK], f16, tag="rmn")
                for ch in range(NCH):
                    c0 = ch * CHUNK
                    ps = psump.tile([128, CHUNK], f32, tag="ps")
                    for sub in range(CHUNK // 512):
                        s0 = sub * 512
                        for k in range(2):
                            nc.tensor.matmul(
                                ps[:, s0:s0 + 512],
                                er_sb[k][:, r0:r0 + 128],
                                e_sb[k][:, c0 + s0:c0 + s0 + 512],
                                start=(k == 0),
                                stop=(k == 1),
                            )
                    qp = chunks.tile([128, CHUNK], f16, tag="qp")
                    nc.scalar.activation(qp, ps, AF.Square, bias=-1.0, scale=1.0)
                    v = chunks.tile([128, CHUNK], f16, tag="v")
                    nc.scalar.activation(v, ps, AF.Relu, bias=0.25, scale=1.0)

                    pos_t = maskp.tile([128, CHUNK], f16, tag="pos")
                    nc.sync.dma_start(out=pos_t, in_=posm[r0:r0 + 128, c0:c0 + CHUNK])
                    neg_t = maskp.tile([128, CHUNK], f16, tag="neg")
                    nc.sync.dma_start(out=neg_t, in_=negm[r0:r0 + 128, c0:c0 + CHUNK])

                    ap_c = arowp.tile([128, CHUNK], f16, tag="ap")
                    an_c = arowp.tile([128, CHUNK], f16, tag="an")
                    ap_cs.append(ap_c)
                    an_cs.append(an_c)
                    if disjoint:
                        # masks disjoint: nw|pos = a+B, nw|neg = b+B
                        aB = chunks.tile([128, CHUNK], f16, tag="aB")
                        nc.vector.tensor_scalar_add(aB, qp, B_SHIFT - 0.0625)
                        nc.vector.tensor_tensor(out=ap_c, in0=aB, in1=pos_t, op=ALU.mult)
                        bB = aB  # dead after ap_c
                        if ACT_ROUTE[ch]:
                            qvn = chunks.tile([128, CHUNK], f16, tag="qvn")
                            nc.scalar.activation(qvn, v, AF.Square, bias=-0.25, scale=1.0)
                            nc.vector.tensor_scalar_add(bB, qvn, B_SHIFT - 0.0625)
                        else:
                            # fn_hat = (v-1/2)*v; bB = fn_hat + B
                            t5 = chunks.tile([128, CHUNK], f16, tag="t5")
                            nc.vector.tensor_scalar_add(t5, v, -0.5)
                            u5 = chunks.tile([128, CHUNK], f16, tag="u5")
                            nc.vector.tensor_tensor(out=u5, in0=t5, in1=v, op=ALU.mult)
                            nc.vector.tensor_scalar_add(bB, u5, B_SHIFT)
                        if no_gp:
                            nc.vector.tensor_tensor(out=an_c, in0=bB, in1=neg_t, op=ALU.mult)
                        else:
                            nc.gpsimd.tensor_tensor(out=an_c, in0=bB, in1=neg_t, op=ALU.mult)
                    else:
                        # general: a = qp - 1/16, t1m = a*pos
                        a_t = chunks.tile([128, CHUNK], f16, tag="a_t")
                        nc.vector.tensor_scalar_add(a_t, qp, -0.0625)
                        pp = chunks.tile([128, CHUNK], f16, tag="pp")
                        nc.vector.tensor_tensor(out=pp, in0=a_t, in1=pos_t, op=ALU.mult)
                        fnm = qp  # reuse dead qp slot
                        qvn = chunks.tile([128, CHUNK], f16, tag="qvn")
                        nc.scalar.activation(qvn, v, AF.Square, bias=-0.25, scale=1.0)
                        b_t = a_t  # dead after pp
                        nc.vector.tensor_scalar_add(b_t, qvn, -0.0625)
                        nc.vector.tensor_tensor(out=fnm, in0=b_t, in1=neg_t, op=ALU.mult)
                        # nw = (pp + fnm) + B
                        s_t = a_t
                        nc.vector.tensor_tensor(out=s_t, in0=pp, in1=fnm, op=ALU.add)
                        nw = v  # reuse dead v slot
                        nc.vector.tensor_scalar_add(nw, s_t, B_SHIFT)
                        nc.vector.tensor_tensor(out=ap_c, in0=nw, in1=pos_t, op=ALU.mult)
                        if no_gp:
                            nc.vector.tensor_tensor(out=an_c, in0=nw, in1=neg_t, op=ALU.mult)
                        else:
                            nc.gpsimd.tensor_tensor(out=an_c, in0=nw, in1=neg_t, op=ALU.mult)
                    # running chunk-wise max
                    if ch == 0:
                        nc.vector.tensor_copy(out=rmp, in_=ap_c)
                        nc.vector.tensor_copy(out=rmn, in_=an_c)
                    else:
                        nc.vector.tensor_tensor(out=rmp, in0=rmp, in1=ap_c, op=ALU.max)
                        nc.vector.tensor_tensor(out=rmn, in0=rmn, in1=an_c, op=ALU.max)

                tail_prio = tc.high_priority(offset=-70)
                tail_prio.__enter__()
                mp = mp_all[:, rt:rt + 1]
                nc.vector.reduce_max(mp, rmp[:, :], axis=mybir.AxisListType.X)
                mn = mn_all[:, rt:rt + 1]
                nc.vector.reduce_max(mn, rmn[:, :], axis=mybir.AxisListType.X)
                bias_p = small.tile([128, 1], f32, tag="bias_p")
                nc.vector.tensor_scalar_mul(bias_p, mp, -256.0)
                bias_n = small.tile([128, 1], f32, tag="bias_n")
                nc.vector.tensor_scalar_mul(bias_n, mn, -256.0)
                # per-chunk in-place exp with fused row-sum parts
                sp_parts = small.tile([128, NCH], f32, tag="sp_parts")
                sn_parts = small.tile([128, NCH], f32, tag="sn_parts")
                for ch in range(NCH if not no_exp else 0):
                    nc.scalar.activation(
                        ap_cs[ch], ap_cs[ch], AF.Exp, bias=bias_p[:, :], scale=256.0,
                        accum_out=sp_parts[:, ch:ch + 1],
                    )
                    nc.scalar.activation(
                        an_cs[ch], an_cs[ch], AF.Exp, bias=bias_n[:, :], scale=256.0,
                        accum_out=sn_parts[:, ch:ch + 1],
                    )
                if not no_exp:
                    nc.vector.reduce_sum(
                        sp_all[:, rt:rt + 1], sp_parts[:, :], axis=mybir.AxisListType.X
                    )
                    nc.vector.reduce_sum(
                        sn_all[:, rt:rt + 1], sn_parts[:, :], axis=mybir.AxisListType.X
                    )
                    tail_prio.__exit__(None, None, None)
                else:
                    nc.vector.tensor_copy(out=sp_all[:, rt:rt + 1], in_=bias_p)
                    nc.vector.tensor_copy(out=sn_all[:, rt:rt + 1], in_=bias_n)
                    tail_prio.__exit__(None, None, None)

            # epilogue on [128, RT]
            lp = small.tile([128, RT], f32, tag="lp")
            nc.scalar.activation(lp, sp_all, AF.Ln, bias=0.0, scale=1.0)
            ln_ = small.tile([128, RT], f32, tag="ln")
            nc.scalar.activation(ln_, sn_all, AF.Ln, bias=0.0, scale=1.0)
            msum = small.tile([128, RT], f32, tag="msum")
            nc.vector.tensor_tensor(out=msum, in0=mp_all, in1=mn_all, op=ALU.add)
            m256 = small.tile([128, RT], f32, tag="m256")
            nc.vector.tensor_scalar(
                m256, msum, -2.0 * B_SHIFT, 256.0, ALU.add, ALU.mult
            )
            lsum = small.tile([128, RT], f32, tag="lsum")
            nc.vector.tensor_tensor(out=lsum, in0=lp, in1=ln_, op=ALU.add)
            lse = small.tile([128, RT], f32, tag="lse")
            nc.vector.tensor_tensor(out=lse, in0=m256, in1=lsum, op=ALU.add)
            # softplus(x) = max(x,0) + ln(1 + exp(-|x|))
            ax = small.tile([128, RT], f32, tag="ax")
            nc.scalar.activation(ax, lse, AF.Abs, bias=0.0, scale=1.0)
            et = small.tile([128, RT], f32, tag="et")
            nc.scalar.activation(et, ax, AF.Exp, bias=0.0, scale=-1.0)
            l1p = small.tile([128, RT], f32, tag="l1p")
            nc.scalar.activation(l1p, et, AF.Ln, bias=1.0, scale=1.0)
            rx = small.tile([128, RT], f32, tag="rx")
            nc.vector.tensor_scalar(rx, lse, 0.0, None, ALU.max)
            loss_t = small.tile([128, RT], f32, tag="loss")
            nc.vector.tensor_tensor(out=loss_t, in0=rx, in1=l1p, op=ALU.add)
            nc.sync.dma_start(out=loss_out[:, :], in_=loss_t)

    _split_waits(nc)
    return nc


def kernel(embeddings: np.ndarray, pos_mask: np.ndarray, neg_mask: np.ndarray) -> np.ndarray:
    import ml_dtypes
    from concourse.bass_utils import run_bass_kernel_spmd

    disjoint = not bool(np.any(np.logical_and(np.asarray(pos_mask), np.asarray(neg_mask))))
    key = "nc_disjoint" if disjoint else "nc_general"
    if key not in _NC_CACHE:
        _NC_CACHE[key] = _build_nc(disjoint=disjoint)
    nc = _NC_CACHE[key]

    emb = np.asarray(embeddings, dtype=np.float32)
    e = emb / np.linalg.norm(emb.astype(np.float64), axis=1, keepdims=True)
    eT = np.ascontiguousarray(e.T).astype(np.float16)

    pos_f16 = np.asarray(pos_mask).astype(np.float16)
    neg_f16 = np.asarray(neg_mask).astype(np.float16)

    in_maps = []
    for c in range(N_CORES):
        r0 = c * ROWS_PER_CORE
        in_maps.append({
            "eT": eT,
            "erT": np.ascontiguousarray(eT[:, r0:r0 + ROWS_PER_CORE]),
            "posm": np.ascontiguousarray(pos_f16[r0:r0 + ROWS_PER_CORE]),
            "negm": np.ascontiguousarray(neg_f16[r0:r0 + ROWS_PER_CORE]),
        })

    res = run_bass_kernel_spmd(
        nc, in_maps, core_ids=list(range(N_CORES)), **_RUN_KWARGS
    )
    _NC_CACHE["last_result"] = res

    losses = np.empty(TWO_N, dtype=np.float32)
    for c in range(N_CORES):
        blk = res.results[c]["loss"]  # [128, RT], loss[p, rt] = row rt*128+p
        losses[c * ROWS_PER_CORE:(c + 1) * ROWS_PER_CORE] = blk.T.reshape(-1)

    valid = np.asarray(pos_mask).any(axis=1) & np.asarray(neg_mask).any(axis=1)
    losses = losses * valid.astype(np.float32)
    nz = losses > 0
    cnt = int(nz.sum())
    if cnt == 0:
        return np.zeros((), dtype=np.float32)
    mean = np.float32(losses.sum(dtype=np.float32) / np.float32(max(cnt, 1)))
    return np.asarray(mean, dtype=np.float32)



# revision 5
# speedup vs baseline: 3.4763x; 3.4763x over previous
"""Circle Loss (PML-style) on 8 Trainium2 NeuronCores via Bass/Tile.

Full inputs -> full scalar output.

Fast path (structured masks, as produced by the reference's setup_inputs:
pos pairs (i, i+N) mod 2N, neg = all except pair and diagonal):
  Row-sharded. Each core computes D = e_rows @ e_all^T for its 1024 rows
  via fp8(e4m3) DoubleRow matmuls, q = max(D,-1/4)*D in one DVE/Pool
  scalar_tensor_tensor pass (PSUM f32 -> SBUF fp16), corrupts the two
  excluded diagonal blocks (self + pair) with -100, then one ACT pass
  exp(256*q - 20) with fused per-row accumulation = masked sum of
  exp(fn - 4), fn = 256*(max(D,-1/4)^2 - 1/16).  No masks are DMA'd at
  all: each core's copy of e^T is column-rotated by its row offset so
  the excluded diagonals sit at build-time-constant offsets.
  Host: lse_n = ln(S) + 4; lse_p = fp(Dp) computed exactly from the
  normalized embeddings; loss = softplus(lse_p + lse_n); mean.

Fallback (any other mask pattern): the original general kernel (masks
DMA'd as fp16, masked logsumexps on device).
"""

import sys

sys.path.insert(0, "/opt/trn_rl_repo")

import numpy as np

TWO_N = 8192
D_EMB = 256
N_CORES = 8
ROWS_PER_CORE = TWO_N // N_CORES  # 1024
RT = ROWS_PER_CORE // 128  # 8 row tiles per core
CHUNK = 2048
NCH = TWO_N // CHUNK  # 4 column chunks
B_SHIFT = 0.75
# fast path: exp(256*q + EXP_BIAS), lse_n = ln(S) + LSE_SHIFT
EXP_BIAS = -20.0
LSE_SHIFT = 4.0
# fraction of (rt, ch) pass-A chunks routed to the Pool engine (rest DVE)
ACT_ROUTE = (True, False, True, False)

_RUN_KWARGS: dict = {}
_NC_CACHE: dict = {}


def _split_waits(nc, maxw=1):
    """walrus in this container accepts at most ~2 sem-waits per
    instruction (1 for ACTIVATE); split extras onto preceding NoOps."""
    import concourse.mybir as mybir

    n_new = 0
    for bb in nc.main_func.blocks:
        insts = bb.instructions
        i = 0
        while i < len(insts):
            ins = insts[i]
            si = ins.sync_info
            if si is not None and si.on_wait and len(si.on_wait) > maxw:
                waits = list(si.on_wait)
                ins.sync_info = mybir.SyncInfo(
                    on_wait=waits[:maxw], on_update=si.on_update
                )
                rest = waits[maxw:]
                pos = i
                while rest:
                    chunk, rest = rest[:maxw], rest[maxw:]
                    nop = mybir.InstNoOp(name=f"I-waitfix-{n_new}")
                    n_new += 1
                    nop.engine = ins.engine
                    nop.sync_info = mybir.SyncInfo(on_wait=chunk, on_update=[])
                    insts.insert(pos, nop)
                    pos += 1
                    i += 1
            i += 1
    return n_new


def _build_nc_fast():
    """Structured-mask fast kernel; one core's program (SPMD)."""
    import concourse.bass as bass
    import concourse.tile as tile
    from concourse import mybir

    f32 = mybir.dt.float32
    f16 = mybir.dt.float16
    fp8 = mybir.dt.float8e4
    AF = mybir.ActivationFunctionType
    ALU = mybir.AluOpType

    nc = bass.Bass("TRN2", target_bir_lowering=False)

    def reg_const(val, dtype=f32):
        t = nc.alloc_sbuf_tensor(f"const-{dtype.name}-{val}", [128, 1], dtype)
        nc.gpsimd.memset(t.ap(), val)
        nc.const_aps.aps[(dtype, val)] = t.ap()

    reg_const(EXP_BIAS)
    nc.all_engine_barrier()

    # e8[p, k, j] = e_norm[(1024*core + j) % 8192, 128*k + p] in fp8e4
    e8 = nc.dram_tensor("e8", [128, 2, TWO_N], fp8, kind="ExternalInput")
    negd = nc.dram_tensor("negd", [128, 128], f16, kind="ExternalInput")
    s_out = nc.dram_tensor("sparts", [128, RT], f32, kind="ExternalOutput")

    # chunk types: A = DVE max-drain + DVE square, B = DVE max-drain +
    # Pool square, C = ACT Square (no clip; D < -1/4 bias is ~4e-4 rel).
    TYPES = []
    cnt = {"A": 8, "B": 17, "C": 7}
    acc = {"A": 0.0, "B": 0.0, "C": 0.0}
    for _ in range(32):
        k = max(cnt, key=lambda t: cnt[t] / 32.0 * (_ + 1) - acc[t])
        TYPES.append(k)
        acc[k] += 1.0

    with tile.TileContext(nc) as tc:
        with tc.tile_pool(name="singles", bufs=1) as singles, \
             tc.tile_pool(name="qpool", bufs=2) as qpool, \
             tc.tile_pool(name="upool", bufs=3) as upool, \
             tc.tile_pool(name="small", bufs=2) as small, \
             tc.tile_pool(name="psum", bufs=2, space="PSUM") as psump:

            e_sb = singles.tile([128, 2, TWO_N], fp8, tag="e8")
            for ch in range(NCH):
                c0 = ch * CHUNK
                nc.sync.dma_start(
                    out=e_sb[:, :, c0:c0 + CHUNK], in_=e8[:, :, c0:c0 + CHUNK]
                )
            nd_sb = singles.tile([128, 128], f16, tag="negd")
            nc.sync.dma_start(out=nd_sb, in_=negd[:, :])
            sp_sb = singles.tile([128, RT], f32, tag="sp")

            for rt in range(RT):
                r0 = rt * 128
                q = qpool.tile([128, TWO_N], f16, tag="q")
                for ch in range(NCH):
                    c0 = ch * CHUNK
                    ps = psump.tile([128, CHUNK], f32, tag="ps")
                    for s in range(CHUNK // 512):
                        s0 = s * 512
                        nc.tensor.matmul(
                            ps[:, s0:s0 + 512],
                            e_sb[:, :, r0:r0 + 128],
                            e_sb[:, :, c0 + s0:c0 + s0 + 512],
                            start=True,
                            stop=True,
                            perf_mode=mybir.MatmulPerfMode.DoubleRow,
                        )
                    typ = TYPES[rt * NCH + ch]
                    qs = q[:, c0:c0 + CHUNK]
                    if typ == "C":
                        nc.scalar.activation(qs, ps, AF.Square, bias=0.0,
                                             scale=1.0)
                    else:
                        u = upool.tile([128, CHUNK], f16, tag="u")
                        nc.vector.tensor_scalar(u, ps, -0.25, None, ALU.max)
                        eng = nc.gpsimd if typ == "B" else nc.vector
                        eng.tensor_tensor(out=qs, in0=u, in1=u, op=ALU.mult)
                # corrupt excluded diagonals: self at col r0, pair at 4096+r0
                nc.vector.tensor_tensor(
                    out=q[:, r0:r0 + 128], in0=q[:, r0:r0 + 128],
                    in1=nd_sb, op=ALU.add,
                )
                p0 = TWO_N // 2 + r0
                nc.vector.tensor_tensor(
                    out=q[:, p0:p0 + 128], in0=q[:, p0:p0 + 128],
                    in1=nd_sb, op=ALU.add,
                )
                nc.scalar.activation(
                    q, q, AF.Exp, bias=EXP_BIAS, scale=256.0,
                    accum_out=sp_sb[:, rt:rt + 1],
                )
            nc.sync.dma_start(out=s_out[:, :], in_=sp_sb)

    _split_waits(nc)
    return nc


def _is_structured(pos_mask, neg_mask):
    pos = np.asarray(pos_mask)
    neg = np.asarray(neg_mask)
    if pos.shape != (TWO_N, TWO_N) or neg.shape != (TWO_N, TWO_N):
        return False
    idx = np.arange(TWO_N)
    pair = (idx + TWO_N // 2) % TWO_N
    if not pos[idx, pair].all() or pos.sum() != TWO_N:
        return False
    eye = np.eye(TWO_N, dtype=bool)
    return not (neg ^ (~pos & ~eye)).any()


def _kernel_fast(embeddings):
    import ml_dtypes
    from concourse import mybir
    from concourse.bass_utils import run_bass_kernel_spmd

    if "nc_fast" not in _NC_CACHE:
        _NC_CACHE["nc_fast"] = _build_nc_fast()
    nc = _NC_CACHE["nc_fast"]

    emb = np.asarray(embeddings, dtype=np.float64)
    e = emb / np.linalg.norm(emb, axis=1, keepdims=True)
    eT8 = np.ascontiguousarray(e.T).astype(np.float32).astype(
        mybir.dt.np(mybir.dt.float8e4)
    )  # [256, 8192]

    negd = (np.eye(128, dtype=np.float32) * -100.0).astype(np.float16)

    in_maps = []
    for c in range(N_CORES):
        rolled = np.roll(eT8, -ROWS_PER_CORE * c, axis=1)
        e8 = np.ascontiguousarray(
            rolled.reshape(2, 128, TWO_N).transpose(1, 0, 2)
        )
        in_maps.append({"e8": e8, "negd": negd})

    res = run_bass_kernel_spmd(
        nc, in_maps, core_ids=list(range(N_CORES)), **_RUN_KWARGS
    )
    _NC_CACHE["last_result"] = res

    S = np.empty(TWO_N, dtype=np.float64)
    for c in range(N_CORES):
        blk = res.results[c]["sparts"]  # [128, RT]; S[1024c+128rt+p]
        S[c * ROWS_PER_CORE:(c + 1) * ROWS_PER_CORE] = \
            blk.astype(np.float64).T.reshape(-1)

    idx = np.arange(TWO_N)
    pair = (idx + TWO_N // 2) % TWO_N
    Dp = np.sum(e * e[pair], axis=1)
    fp = 256.0 * ((Dp - 1.0) ** 2 - 1.0 / 16.0)
    lse = fp + np.log(np.maximum(S, 1e-300)) + LSE_SHIFT
    losses = np.log1p(np.exp(-np.abs(lse))) + np.maximum(lse, 0.0)
    losses = losses.astype(np.float32)
    nz = losses > 0
    cnt = int(nz.sum())
    if cnt == 0:
        return np.zeros((), dtype=np.float32)
    mean = np.float32(losses.sum(dtype=np.float64) / max(cnt, 1))
    return np.asarray(mean, dtype=np.float32)


# ---------------------------------------------------------------------------
# general fallback (original kernel): masks DMA'd, masked logsumexps on device
# ---------------------------------------------------------------------------

def _build_nc(disjoint=False):
    import os
    import concourse.bass as bass
    import concourse.tile as tile
    from concourse import mybir

    no_gp = os.environ.get("K_NOGP", "0") == "1"
    no_exp = os.environ.get("K_NOEXP", "0") == "1"
    repeat = int(os.environ.get("K_REPEAT", "1"))

    f32 = mybir.dt.float32
    f16 = mybir.dt.float16
    AF = mybir.ActivationFunctionType
    ALU = mybir.AluOpType

    nc = bass.Bass("TRN2", target_bir_lowering=False)

    def reg_const(val, dtype=f32):
        t = nc.alloc_sbuf_tensor(f"const-{dtype.name}-{val}", [128, 1], dtype)
        nc.gpsimd.memset(t.ap(), val)
        nc.const_aps.aps[(dtype, val)] = t.ap()

    for vv in (-1.0, 0.25, -0.25):
        reg_const(vv)
    nc.all_engine_barrier()

    eT = nc.dram_tensor("eT", [D_EMB, TWO_N], f16, kind="ExternalInput")
    erT = nc.dram_tensor("erT", [D_EMB, ROWS_PER_CORE], f16, kind="ExternalInput")
    posm = nc.dram_tensor("posm", [ROWS_PER_CORE, TWO_N], f16, kind="ExternalInput")
    negm = nc.dram_tensor("negm", [ROWS_PER_CORE, TWO_N], f16, kind="ExternalInput")
    loss_out = nc.dram_tensor("loss", [128, RT], f32, kind="ExternalOutput")

    with tile.TileContext(nc) as tc:
        with tc.tile_pool(name="singles", bufs=1) as singles, \
             tc.tile_pool(name="chunks", bufs=3) as chunks, \
             tc.tile_pool(name="masks", bufs=2) as maskp, \
             tc.tile_pool(name="arow", bufs=6) as arowp, \
             tc.tile_pool(name="rmax", bufs=2) as rmaxp, \
             tc.tile_pool(name="small", bufs=4) as small, \
             tc.tile_pool(name="psum", bufs=2, space="PSUM") as psump:

            e_sb = []
            er_sb = []
            for k in range(2):
                t = singles.tile([128, TWO_N], f16, tag=f"e{k}")
                nc.sync.dma_start(out=t, in_=eT[k * 128:(k + 1) * 128, :])
                e_sb.append(t)
                tr = singles.tile([128, ROWS_PER_CORE], f16, tag=f"er{k}")
                nc.sync.dma_start(out=tr, in_=erT[k * 128:(k + 1) * 128, :])
                er_sb.append(tr)

            sp_all = singles.tile([128, RT], f32, tag="sp_all")
            sn_all = singles.tile([128, RT], f32, tag="sn_all")
            mp_all = singles.tile([128, RT], f32, tag="mp_all")
            mn_all = singles.tile([128, RT], f32, tag="mn_all")

            for rep in range(repeat):
              for rt in range(RT):
                r0 = rt * 128
                ap_cs = []
                an_cs = []
                rmp = rmaxp.tile([128, CHUNK], f16, tag="rmp")
                rmn = rmaxp.tile([128, CHUNK], f16, tag="rmn")
                for ch in range(NCH):
                    c0 = ch * CHUNK
                    ps = psump.tile([128, CHUNK], f32, tag="ps")
                    for sub in range(CHUNK // 512):
                        s0 = sub * 512
                        for k in range(2):
                            nc.tensor.matmul(
                                ps[:, s0:s0 + 512],
                                er_sb[k][:, r0:r0 + 128],
                                e_sb[k][:, c0 + s0:c0 + s0 + 512],
                                start=(k == 0),
                                stop=(k == 1),
                            )
                    qp = chunks.tile([128, CHUNK], f16, tag="qp")
                    nc.scalar.activation(qp, ps, AF.Square, bias=-1.0, scale=1.0)
                    v = chunks.tile([128, CHUNK], f16, tag="v")
                    nc.scalar.activation(v, ps, AF.Relu, bias=0.25, scale=1.0)

                    pos_t = maskp.tile([128, CHUNK], f16, tag="pos")
                    nc.sync.dma_start(out=pos_t, in_=posm[r0:r0 + 128, c0:c0 + CHUNK])
                    neg_t = maskp.tile([128, CHUNK], f16, tag="neg")
                    nc.sync.dma_start(out=neg_t, in_=negm[r0:r0 + 128, c0:c0 + CHUNK])

                    ap_c = arowp.tile([128, CHUNK], f16, tag="ap")
                    an_c = arowp.tile([128, CHUNK], f16, tag="an")
                    ap_cs.append(ap_c)
                    an_cs.append(an_c)
                    if disjoint:
                        # masks disjoint: nw|pos = a+B, nw|neg = b+B
                        aB = chunks.tile([128, CHUNK], f16, tag="aB")
                        nc.vector.tensor_scalar_add(aB, qp, B_SHIFT - 0.0625)
                        nc.vector.tensor_tensor(out=ap_c, in0=aB, in1=pos_t, op=ALU.mult)
                        bB = aB  # dead after ap_c
                        if ACT_ROUTE[ch]:
                            qvn = chunks.tile([128, CHUNK], f16, tag="qvn")
                            nc.scalar.activation(qvn, v, AF.Square, bias=-0.25, scale=1.0)
                            nc.vector.tensor_scalar_add(bB, qvn, B_SHIFT - 0.0625)
                        else:
                            # fn_hat = (v-1/2)*v; bB = fn_hat + B
                            t5 = chunks.tile([128, CHUNK], f16, tag="t5")
                            nc.vector.tensor_scalar_add(t5, v, -0.5)
                            u5 = chunks.tile([128, CHUNK], f16, tag="u5")
                            nc.vector.tensor_tensor(out=u5, in0=t5, in1=v, op=ALU.mult)
                            nc.vector.tensor_scalar_add(bB, u5, B_SHIFT)
                        if no_gp:
                            nc.vector.tensor_tensor(out=an_c, in0=bB, in1=neg_t, op=ALU.mult)
                        else:
                            nc.gpsimd.tensor_tensor(out=an_c, in0=bB, in1=neg_t, op=ALU.mult)
                    else:
                        # general: a = qp - 1/16, t1m = a*pos
                        a_t = chunks.tile([128, CHUNK], f16, tag="a_t")
                        nc.vector.tensor_scalar_add(a_t, qp, -0.0625)
                        pp = chunks.tile([128, CHUNK], f16, tag="pp")
                        nc.vector.tensor_tensor(out=pp, in0=a_t, in1=pos_t, op=ALU.mult)
                        fnm = qp  # reuse dead qp slot
                        qvn = chunks.tile([128, CHUNK], f16, tag="qvn")
                        nc.scalar.activation(qvn, v, AF.Square, bias=-0.25, scale=1.0)
                        b_t = a_t  # dead after pp
                        nc.vector.tensor_scalar_add(b_t, qvn, -0.0625)
                        nc.vector.tensor_tensor(out=fnm, in0=b_t, in1=neg_t, op=ALU.mult)
                        # nw = (pp + fnm) + B
                        s_t = a_t
                        nc.vector.tensor_tensor(out=s_t, in0=pp, in1=fnm, op=ALU.add)
                        nw = v  # reuse dead v slot
                        nc.vector.tensor_scalar_add(nw, s_t, B_SHIFT)
                        nc.vector.tensor_tensor(out=ap_c, in0=nw, in1=pos_t, op=ALU.mult)
                        if no_gp:
                            nc.vector.tensor_tensor(out=an_c, in0=nw, in1=neg_t, op=ALU.mult)
                        else:
                            nc.gpsimd.tensor_tensor(out=an_c, in0=nw, in1=neg_t, op=ALU.mult)
                    # running chunk-wise max
                    if ch == 0:
                        nc.vector.tensor_copy(out=rmp, in_=ap_c)
                        nc.vector.tensor_copy(out=rmn, in_=an_c)
                    else:
                        nc.vector.tensor_tensor(out=rmp, in0=rmp, in1=ap_c, op=ALU.max)
                        nc.vector.tensor_tensor(out=rmn, in0=rmn, in1=an_c, op=ALU.max)

                tail_prio = tc.high_priority(offset=-70)
                tail_prio.__enter__()
                mp = mp_all[:, rt:rt + 1]
                nc.vector.reduce_max(mp, rmp[:, :], axis=mybir.AxisListType.X)
                mn = mn_all[:, rt:rt + 1]
                nc.vector.reduce_max(mn, rmn[:, :], axis=mybir.AxisListType.X)
                bias_p = small.tile([128, 1], f32, tag="bias_p")
                nc.vector.tensor_scalar_mul(bias_p, mp, -256.0)
                bias_n = small.tile([128, 1], f32, tag="bias_n")
                nc.vector.tensor_scalar_mul(bias_n, mn, -256.0)
                # per-chunk in-place exp with fused row-sum parts
                sp_parts = small.tile([128, NCH], f32, tag="sp_parts")
                sn_parts = small.tile([128, NCH], f32, tag="sn_parts")
                for ch in range(NCH if not no_exp else 0):
                    nc.scalar.activation(
                        ap_cs[ch], ap_cs[ch], AF.Exp, bias=bias_p[:, :], scale=256.0,
                        accum_out=sp_parts[:, ch:ch + 1],
                    )
                    nc.scalar.activation(
                        an_cs[ch], an_cs[ch], AF.Exp, bias=bias_n[:, :], scale=256.0,
                        accum_out=sn_parts[:, ch:ch + 1],
                    )
                if not no_exp:
                    nc.vector.reduce_sum(
                        sp_all[:, rt:rt + 1], sp_parts[:, :], axis=mybir.AxisListType.X
                    )
                    nc.vector.reduce_sum(
                        sn_all[:, rt:rt + 1], sn_parts[:, :], axis=mybir.AxisListType.X
                    )
                    tail_prio.__exit__(None, None, None)
                else:
                    nc.vector.tensor_copy(out=sp_all[:, rt:rt + 1], in_=bias_p)
                    nc.vector.tensor_copy(out=sn_all[:, rt:rt + 1], in_=bias_n)
                    tail_prio.__exit__(None, None, None)

            # epilogue on [128, RT]
            lp = small.tile([128, RT], f32, tag="lp")
            nc.scalar.activation(lp, sp_all, AF.Ln, bias=0.0, scale=1.0)
            ln_ = small.tile([128, RT], f32, tag="ln")
            nc.scalar.activation(ln_, sn_all, AF.Ln, bias=0.0, scale=1.0)
            msum = small.tile([128, RT], f32, tag="msum")
            nc.vector.tensor_tensor(out=msum, in0=mp_all, in1=mn_all, op=ALU.add)
            m256 = small.tile([128, RT], f32, tag="m256")
            nc.vector.tensor_scalar(
                m256, msum, -2.0 * B_SHIFT, 256.0, ALU.add, ALU.mult
            )
            lsum = small.tile([128, RT], f32, tag="lsum")
            nc.vector.tensor_tensor(out=lsum, in0=lp, in1=ln_, op=ALU.add)
            lse = small.tile([128, RT], f32, tag="lse")
            nc.vector.tensor_tensor(out=lse, in0=m256, in1=lsum, op=ALU.add)
            # softplus(x) = max(x,0) + ln(1 + exp(-|x|))
            ax = small.tile([128, RT], f32, tag="ax")
            nc.scalar.activation(ax, lse, AF.Abs, bias=0.0, scale=1.0)
            et = small.tile([128, RT], f32, tag="et")
            nc.scalar.activation(et, ax, AF.Exp, bias=0.0, scale=-1.0)
            l1p = small.tile([128, RT], f32, tag="l1p")
            nc.scalar.activation(l1p, et, AF.Ln, bias=1.0, scale=1.0)
            rx = small.tile([128, RT], f32, tag="rx")
            nc.vector.tensor_scalar(rx, lse, 0.0, None, ALU.max)
            loss_t = small.tile([128, RT], f32, tag="loss")
            nc.vector.tensor_tensor(out=loss_t, in0=rx, in1=l1p, op=ALU.add)
            nc.sync.dma_start(out=loss_out[:, :], in_=loss_t)

    _split_waits(nc)
    return nc


def _kernel_general(embeddings, pos_mask, neg_mask):
    from concourse.bass_utils import run_bass_kernel_spmd

    disjoint = not bool(np.any(np.logical_and(np.asarray(pos_mask), np.asarray(neg_mask))))
    key = "nc_disjoint" if disjoint else "nc_general"
    if key not in _NC_CACHE:
        _NC_CACHE[key] = _build_nc(disjoint=disjoint)
    nc = _NC_CACHE[key]

    emb = np.asarray(embeddings, dtype=np.float32)
    e = emb / np.linalg.norm(emb.astype(np.float64), axis=1, keepdims=True)
    eT = np.ascontiguousarray(e.T).astype(np.float16)

    pos_f16 = np.asarray(pos_mask).astype(np.float16)
    neg_f16 = np.asarray(neg_mask).astype(np.float16)

    in_maps = []
    for c in range(N_CORES):
        r0 = c * ROWS_PER_CORE
        in_maps.append({
            "eT": eT,
            "erT": np.ascontiguousarray(eT[:, r0:r0 + ROWS_PER_CORE]),
            "posm": np.ascontiguousarray(pos_f16[r0:r0 + ROWS_PER_CORE]),
            "negm": np.ascontiguousarray(neg_f16[r0:r0 + ROWS_PER_CORE]),
        })

    res = run_bass_kernel_spmd(
        nc, in_maps, core_ids=list(range(N_CORES)), **_RUN_KWARGS
    )
    _NC_CACHE["last_result"] = res

    losses = np.empty(TWO_N, dtype=np.float32)
    for c in range(N_CORES):
        blk = res.results[c]["loss"]  # [128, RT], loss[p, rt] = row rt*128+p
        losses[c * ROWS_PER_CORE:(c + 1) * ROWS_PER_CORE] = blk.T.reshape(-1)

    valid = np.asarray(pos_mask).any(axis=1) & np.asarray(neg_mask).any(axis=1)
    losses = losses * valid.astype(np.float32)
    nz = losses > 0
    cnt = int(nz.sum())
    if cnt == 0:
        return np.zeros((), dtype=np.float32)
    mean = np.float32(losses.sum(dtype=np.float32) / np.float32(max(cnt, 1)))
    return np.asarray(mean, dtype=np.float32)


def kernel(embeddings: np.ndarray, pos_mask: np.ndarray, neg_mask: np.ndarray) -> np.ndarray:
    if _is_structured(pos_mask, neg_mask):
        return _kernel_fast(embeddings)
    return _kernel_general(embeddings, pos_mask, neg_mask)
